# revision 17
# baseline (speedup 1.0000x reference)
"""Trainium2 Bass kernel for nn_Attention_40561671144003.

Head-parallel sharding: 8 heads -> 8 NeuronCores, one head per core.
Each core computes its head's q/k/v projections (reading the full
replicated activations), full-sequence attention for that head, and a
partial output projection.  The host sums the 8 partial projections
(the "all-reduce after proj" step) and overlays each core's
vflat-contribution rows.

Math per core (head h):
  xT   = concat(query, value, axis=-1).T                      [1024, 4096]
  qT/kT/vT = w_{q,k,v}[h] @ xT                                [64, 4096]
  sT   = kT.T @ qT        (scores, transposed: [m, n])
  pT   = exp(sT / 8)      (no max-subtraction: |s/8| <= ~9)
  o_aug= pT.T @ [v | 1]   -> o = o_aug[:, :64] / o_aug[:, 64] [n, 64]
  ypartT = (o @ w_proj[:, h*64:(h+1)*64].T).T                 [1024, 4096]
  yvout  = v.reshape(512, 512) @ w_proj[:, 512:].T + b_proj   [512, 1024]
  host: y = sum_h ypartT_h.T;  y[h*512:(h+1)*512] += yvout_h

Performance notes (measured on trn2):
  - fp32 matmul streams at ~2 cycles/column and fp32 LDWEIGHTS costs a
    full ~512 cycles; 16-bit operands stream 1 col/cycle and get fast
    weight load.  All matmul operands are fp16 (not bf16 - fp16 keeps
    ~5e-4 relative precision); every accumulation stays in fp32 PSUM.
  - qkv runs over sblk pairs so each weight LDW feeds two MMs; pair-0
    attention groups are interleaved into the qkv phase so ACT (exp)
    never idles.
  - ACT exp (1 elem/lane/cycle @ 1.2GHz + per-instr overhead) is the
    steady-state bound; yo/yv matmuls drip into PE slack via a queue.
"""

from collections import deque
from contextlib import ExitStack

import numpy as np

import concourse.bacc as bacc
import concourse.tile as tile
from concourse import mybir
from concourse.bass_utils import run_bass_kernel_spmd
from concourse.masks import make_identity

FP32 = mybir.dt.float32
FP16 = mybir.dt.float16
Exp = mybir.ActivationFunctionType.Exp

N = 4096          # sequence length
C = 512           # channels
H = 8             # heads
D = 64            # head dim
SB = 512          # sequence block
NSB = N // SB     # 8
NPAIR = NSB // 2  # 4 n-block pairs
MT = N // 128     # 32 m-tiles of 128
SCALE = D ** -0.5


def _emit(ctx, tc, nc, io):
    xT, wqkT, wvT, w1, w2, bvec, ypartT, yvout = io

    consts = ctx.enter_context(tc.tile_pool(name="consts", bufs=1))
    persist = ctx.enter_context(tc.tile_pool(name="persist", bufs=1))
    xpool = ctx.enter_context(tc.tile_pool(name="xpool", bufs=16))
    ppool = ctx.enter_context(tc.tile_pool(name="ppool", bufs=4))
    ypool = ctx.enter_context(tc.tile_pool(name="ypool", bufs=4))
    smalls = ctx.enter_context(tc.tile_pool(name="smalls", bufs=8))
    psA = ctx.enter_context(tc.tile_pool(name="psA", bufs=2, space="PSUM"))
    psO = ctx.enter_context(tc.tile_pool(name="psO", bufs=2, space="PSUM"))
    psS = ctx.enter_context(tc.tile_pool(name="psS", bufs=2, space="PSUM"))

    # ---- constants ----
    wqk_sb = consts.tile([128, 8, 128], FP16, tag="wqk")
    nc.sync.dma_start(out=wqk_sb, in_=wqkT.rearrange("(t p) m -> p t m", p=128))
    wv_sb = consts.tile([128, 8, 64], FP16, tag="wv")
    nc.sync.dma_start(out=wv_sb, in_=wvT.rearrange("(t p) m -> p t m", p=128))
    w1_sb = consts.tile([128, 1024], FP16, tag="w1")
    nc.sync.dma_start(out=w1_sb, in_=w1[:, :])
    w2_sb = consts.tile([128, 4, 1024], FP16, tag="w2")
    nc.sync.dma_start(out=w2_sb, in_=w2.rearrange("(t p) j -> p t j", p=128))
    import concourse.bass as bass
    ident = consts.tile([128, 128], FP16, tag="ident")
    make_identity(nc, ident)
    bvb = consts.tile([128, 1024], FP32, tag="bvb")
    nc.sync.dma_start(
        out=bvb,
        in_=bass.AP(tensor=bvec.tensor, offset=bvec.offset,
                    ap=[[0, 128]] + list(bvec.ap[1:])),
    )

    # ---- persistent activations (all fp16, PSUM accumulation is fp32) ----
    qTb = [persist.tile([128, 1024], FP16, tag=f"qTb{i}", name=f"qTb{i}")
           for i in range(NPAIR)]
    kTb = [persist.tile([128, 1024], FP16, tag=f"kTb{i}", name=f"kTb{i}")
           for i in range(NPAIR)]
    for t in qTb + kTb:
        nc.vector.memset(t[64:128, :], 0.0)
    vT = [persist.tile([64, SB], FP16, tag=f"vT{i}", name=f"vT{i}")
          for i in range(NSB)]
    vaug = [persist.tile([128, 65], FP16, tag=f"va{i}", name=f"va{i}")
            for i in range(MT)]
    oT = [persist.tile([128, SB], FP16, tag=f"oT{i}", name=f"oT{i}")
          for i in range(NSB)]
    for t in oT:
        nc.vector.memset(t[64:128, :], 0.0)
    vflatT = [
        persist.tile([128, 128], FP16, tag=f"vf{i}", name=f"vf{i}")
        for i in range(16)
    ]

    # ---- qkv projection for one sblk pair ----
    def emit_qkv(pr):
        xts = []
        for c in range(8):
            xt = xpool.tile([128, 1024], FP16, name=f"xt{pr}_{c}", tag="xt")
            nc.sync.dma_start(
                out=xt, in_=xT[c * 128:(c + 1) * 128, pr * 1024:(pr + 1) * 1024]
            )
            xts.append(xt)
        qk_ps = psA.tile([128, 1024], FP32, name=f"qkps{pr}", tag="A")
        for c in range(8):
            for half in range(2):
                nc.tensor.matmul(
                    qk_ps[:, half * SB:(half + 1) * SB],
                    lhsT=wqk_sb[:, c, :],
                    rhs=xts[c][:, half * SB:(half + 1) * SB],
                    start=(c == 0), stop=(c == 7),
                )
        for half in range(2):
            sl = slice(half * SB, (half + 1) * SB)
            nc.vector.tensor_copy(qTb[pr][0:64, sl], qk_ps[0:64, sl])
            nc.vector.tensor_copy(kTb[pr][0:64, sl], qk_ps[64:128, sl])
        v_ps = psA.tile([64, 1024], FP32, name=f"vps{pr}", tag="A")
        for c in range(8):
            for half in range(2):
                nc.tensor.matmul(
                    v_ps[:, half * SB:(half + 1) * SB],
                    lhsT=wv_sb[:, c, :],
                    rhs=xts[c][:, half * SB:(half + 1) * SB],
                    start=(c == 0), stop=(c == 7),
                )
        for half in range(2):
            sblk = 2 * pr + half
            nc.vector.tensor_copy(vT[sblk], v_ps[:, half * SB:(half + 1) * SB])
            # v natural layout [m, d] (+ ones col) for the ov matmul
            for j in range(4):
                mt = sblk * 4 + j
                tr_ps = psS.tile([128, 64], FP16, name=f"vtr{mt}", tag="sp")
                nc.tensor.transpose(
                    tr_ps, vT[sblk][:, j * 128:(j + 1) * 128], ident[0:64, 0:64]
                )
                nc.vector.tensor_copy(vaug[mt][:, 0:64], tr_ps)
                nc.vector.memset(vaug[mt][:, 64:65], 1.0)

    # vflatT[c2, r] = v[8r + c2//64, c2%64] (feeds the proj directly)
    def emit_vflat():
        for k2 in range(4):
            for rt in range(4):
                vf = vflatT[k2 * 4 + rt]
                for s01 in range(2):
                    for half in range(2):
                        t = 2 * rt + half
                        src = vT[t].rearrange("p (r s) -> p s r", s=8)
                        nc.vector.tensor_copy(
                            vf[64 * s01:64 * s01 + 64, 64 * half:64 * half + 64],
                            src[:, 2 * k2 + s01, :],
                        )

    # ---- pending PE work (yv, yo) drip-fed into the attention pipeline ----
    pending = deque()

    def queue_yv():
        yvstage = [None] * 4

        yps_live = {}

        def mk_a(rt, half):
            def thunk():
                yps = psS.tile([128, SB], FP32, name=f"yvps{rt}_{half}", tag="sp")
                yps_live[(rt, half)] = yps
                for k2 in range(2):
                    nc.tensor.matmul(
                        yps, lhsT=vflatT[k2 * 4 + rt],
                        rhs=w2_sb[:, k2, half * 512:(half + 1) * 512],
                        start=(k2 == 0), stop=False,
                    )
            return thunk

        def mk_b(rt, half):
            def thunk():
                yps = yps_live.pop((rt, half))
                for k2 in range(2, 4):
                    nc.tensor.matmul(
                        yps, lhsT=vflatT[k2 * 4 + rt],
                        rhs=w2_sb[:, k2, half * 512:(half + 1) * 512],
                        start=False, stop=(k2 == 3),
                    )
                if yvstage[rt] is None:
                    yvstage[rt] = ypool.tile(
                        [128, 1024], FP32, name=f"yvst{rt}", tag="yst"
                    )
                nc.vector.tensor_add(
                    yvstage[rt][:, half * 512:(half + 1) * 512], yps,
                    bvb[:, half * 512:(half + 1) * 512],
                )
                if half == 1:
                    nc.sync.dma_start(
                        out=yvout[rt * 128:(rt + 1) * 128, :], in_=yvstage[rt]
                    )
            return thunk

        for rt in range(4):
            for half in range(2):
                pending.append(mk_a(rt, half))
                pending.append(mk_b(rt, half))

    def queue_yo(pr):
        ystages = {}

        def mk(j, half):
            def thunk():
                if j not in ystages:
                    ystages[j] = ypool.tile(
                        [128, 1024], FP32, name=f"yst{pr}_{j}", tag="yst"
                    )
                ystage = ystages[j]
                nb = 2 * pr + half
                yps = psS.tile([128, SB], FP32, name=f"yops{pr}_{j}_{half}",
                               tag="sp")
                nc.tensor.matmul(
                    yps, lhsT=w1_sb[:, j * 128:(j + 1) * 128], rhs=oT[nb],
                    start=True, stop=True,
                )
                nc.vector.tensor_copy(
                    ystage[:, half * 512:(half + 1) * 512], yps
                )
                if half == 1:
                    nc.sync.dma_start(
                        out=ypartT[j * 128:(j + 1) * 128,
                                   pr * 1024:(pr + 1) * 1024],
                        in_=ystages.pop(j),
                    )
            return thunk

        for j in range(8):
            pending.append(mk(j, 0))
            pending.append(mk(j, 1))

    # ---- attention: dual-nblock sweeps, software-pipelined ----
    groups = [(pr, mt) for pr in range(NPAIR) for mt in range(MT)]
    s_ps = [None] * len(groups)
    p_sb = [None] * len(groups)
    oaug = [None] * NSB

    def emit_s(i):
        pr, mt = groups[i]
        ps = psA.tile([128, 1024], FP32, name=f"sps{pr}_{mt}", tag="A")
        lhsT = kTb[mt // 8][:, (mt % 8) * 128:(mt % 8) * 128 + 128]
        for half in range(2):
            sl = slice(half * SB, (half + 1) * SB)
            nc.tensor.matmul(ps[:, sl], lhsT=lhsT, rhs=qTb[pr][:, sl],
                             start=True, stop=True)
        s_ps[i] = ps

    def emit_exp(i):
        pr, mt = groups[i]
        pt = ppool.tile([128, 1024], FP16, name=f"pt{pr}_{mt}", tag="pt")
        nc.scalar.activation(pt, s_ps[i], Exp, scale=SCALE)
        p_sb[i] = pt

    def emit_ov(i):
        pr, mt = groups[i]
        for half in range(2):
            nb = 2 * pr + half
            if oaug[nb] is None:
                oaug[nb] = psO.tile([128, SB], FP32, name=f"oaug{nb}", tag="oaug")
            for nt in range(4):
                # one accumulation group per PSUM bank: single start/stop,
                # first touch of each byte overwrites via pending-zero bits
                nc.tensor.matmul(
                    oaug[nb][:, nt * 128:nt * 128 + 65],
                    lhsT=p_sb[i][:, half * SB + nt * 128:half * SB + nt * 128 + 128],
                    rhs=vaug[mt],
                    start=(mt == 0 and nt == 0), stop=(mt == MT - 1 and nt == 3),
                )
        p_sb[i] = None
        s_ps[i] = None

    def emit_tail(pr):
        for half in range(2):
            nb = 2 * pr + half
            for nt in range(4):
                sl = oaug[nb][:, nt * 128:nt * 128 + 65]
                rec = smalls.tile([128, 1], FP32, name=f"rec{nb}_{nt}", tag="rec")
                nc.vector.reciprocal(rec, sl[:, 64:65])
                onrm = smalls.tile([128, 64], FP16, name=f"on{nb}_{nt}", tag="onrm")
                nc.vector.tensor_scalar_mul(onrm, sl[:, 0:64], rec)
                tr_ps = psS.tile([64, 128], FP16, name=f"otr{nb}_{nt}", tag="sp")
                nc.tensor.transpose(tr_ps, onrm, ident)
                nc.vector.tensor_copy(oT[nb][0:64, nt * 128:nt * 128 + 128], tr_ps)
            oaug[nb] = None

    # emission schedule: qkv pairs 0-1 up front, pairs 2-3 and the
    # vflat/yv prep interleaved into pair-0's attention sweep
    emit_qkv(0)
    emit_s(0)
    emit_s(1)
    for i in range(len(groups)):
        if i + 2 < len(groups):
            emit_s(i + 2)
        emit_exp(i)
        emit_ov(i)
        if pending:
            pending.popleft()()
        pr, mt = groups[i]
        if pr == 0 and mt == 3:
            emit_qkv(1)
        elif pr == 0 and mt == 7:
            emit_qkv(2)
        elif pr == 0 and mt == 15:
            emit_qkv(3)
        elif pr == 0 and mt == 23:
            emit_vflat()
            queue_yv()
        if mt == MT - 1:
            emit_tail(pr)
            queue_yo(pr)
    while pending:
        pending.popleft()()


def build_program():
    nc = bacc.Bacc()
    xT = nc.declare_dram_parameter("xT", [1024, N], FP16, isOutput=False)
    wqkT = nc.declare_dram_parameter("wqkT", [1024, 128], FP16, isOutput=False)
    wvT = nc.declare_dram_parameter("wvT", [1024, 64], FP16, isOutput=False)
    w1 = nc.declare_dram_parameter("w1", [128, 1024], FP16, isOutput=False)
    w2 = nc.declare_dram_parameter("w2", [512, 1024], FP16, isOutput=False)
    bvec = nc.declare_dram_parameter("bvec", [1, 1024], FP32, isOutput=False)
    ypartT = nc.declare_dram_parameter("ypartT", [1024, N], FP32, isOutput=True)
    yvout = nc.declare_dram_parameter("yvout", [512, 1024], FP32, isOutput=True)
    io = (xT[:], wqkT[:], wvT[:], w1[:], w2[:], bvec[:], ypartT[:], yvout[:])
    with tile.TileContext(nc) as tc:
        with ExitStack() as ctx:
            _emit(ctx, tc, nc, io)
    nc.compile()
    return nc


def make_in_maps(query, value, w_qkv, w_proj, b_proj):
    x = np.concatenate([query[0], value[0]], axis=1).astype(np.float32)
    xT = np.ascontiguousarray(x.T).astype(np.float16)
    w2 = np.ascontiguousarray(w_proj[:, 512:1024].T).astype(np.float16)
    bv = np.ascontiguousarray(b_proj[None, :]).astype(np.float32)
    in_maps = []
    for h in range(H):
        wqk = np.ascontiguousarray(
            np.concatenate(
                [w_qkv[h * 64:(h + 1) * 64], w_qkv[512 + h * 64:512 + (h + 1) * 64]],
                axis=0,
            ).T
        ).astype(np.float16)
        wv = np.ascontiguousarray(
            w_qkv[1024 + h * 64:1024 + (h + 1) * 64].T
        ).astype(np.float16)
        w1 = np.zeros((128, 1024), np.float16)
        w1[:64] = w_proj[:, h * 64:(h + 1) * 64].T.astype(np.float16)
        in_maps.append(
            {"xT": xT, "wqkT": wqk, "wvT": wv, "w1": w1, "w2": w2, "bvec": bv}
        )
    return in_maps


def combine_results(results):
    yT = np.zeros((1024, N), np.float32)
    for h in range(H):
        yT += results[h]["ypartT"]
    y = np.ascontiguousarray(yT.T)
    for h in range(H):
        y[h * 512:(h + 1) * 512] += results[h]["yvout"]
    y0 = np.ascontiguousarray(y[:, :512].reshape(1, N, 512))
    y1 = np.ascontiguousarray(y[:, 512:].reshape(1, N, 512))
    return y0, y1


_PROGRAM = None


def kernel(query, value, w_qkv, w_proj, b_proj, **_):
    global _PROGRAM
    if _PROGRAM is None:
        _PROGRAM = build_program()
    in_maps = make_in_maps(query, value, w_qkv, w_proj, b_proj)
    res = run_bass_kernel_spmd(_PROGRAM, in_maps, list(range(H)))
    return combine_results(res.results)


# revision 18
# speedup vs baseline: 1.1972x; 1.1972x over previous
"""Trainium2 Bass kernel for nn_Attention_40561671144003.

Head-parallel sharding: 8 heads -> 8 NeuronCores, one head per core.
Each core computes its head's q/k/v projections (reading the full
replicated activations), full-sequence attention for that head, and a
partial output projection.  The host sums the 8 partial projections
(the "all-reduce after proj" step) and overlays each core's
vflat-contribution rows.

Math per core (head h):
  xT   = concat(query, value, axis=-1).T                      [1024, 4096]
  qT/kT/vT = w_{q,k,v}[h] @ xT                                [64, 4096]
  sT   = kT.T @ qT        (scores, transposed: [m, n])
  pT   = exp(sT / 8)      (no max-subtraction: |s/8| <= ~9)
  o_aug= pT.T @ [v | 1]   -> o = o_aug[:, :64] / o_aug[:, 64] [n, 64]
  ypartT = (o @ w_proj[:, h*64:(h+1)*64].T).T                 [1024, 4096]
  yvout  = v.reshape(512, 512) @ w_proj[:, 512:].T + b_proj   [512, 1024]
  host: y = sum_h ypartT_h.T;  y[h*512:(h+1)*512] += yvout_h

Performance notes (measured on trn2):
  - fp32 matmul streams at ~2 cycles/column and fp32 LDWEIGHTS costs a
    full ~512 cycles; 16-bit operands stream 1 col/cycle and get fast
    weight load.  All matmul operands are fp16 (not bf16 - fp16 keeps
    ~5e-4 relative precision); every accumulation stays in fp32 PSUM.
  - qkv runs over sblk pairs so each weight LDW feeds two MMs; pair-0
    attention groups are interleaved into the qkv phase so ACT (exp)
    never idles.
  - ACT exp (1 elem/lane/cycle @ 1.2GHz + per-instr overhead) is the
    steady-state bound; yo/yv matmuls drip into PE slack via a queue.
"""

from collections import deque
from contextlib import ExitStack

import numpy as np

import concourse.bacc as bacc
import concourse.tile as tile
from concourse import mybir
from concourse.bass_utils import run_bass_kernel_spmd
from concourse.masks import make_identity

FP32 = mybir.dt.float32
FP16 = mybir.dt.float16
Exp = mybir.ActivationFunctionType.Exp

N = 4096          # sequence length
C = 512           # channels
H = 8             # heads
D = 64            # head dim
SB = 512          # sequence block
NSB = N // SB     # 8
NPAIR = NSB // 2  # 4 n-block pairs
MT = N // 128     # 32 m-tiles of 128
SCALE = D ** -0.5


def _emit(ctx, tc, nc, io):
    xT, wqkT, wvT, w1, w2, bvec, ypartT, yvout = io

    consts = ctx.enter_context(tc.tile_pool(name="consts", bufs=1))
    persist = ctx.enter_context(tc.tile_pool(name="persist", bufs=1))
    xpool = ctx.enter_context(tc.tile_pool(name="xpool", bufs=16))
    ppool = ctx.enter_context(tc.tile_pool(name="ppool", bufs=4))
    ypool = ctx.enter_context(tc.tile_pool(name="ypool", bufs=4))
    smalls = ctx.enter_context(tc.tile_pool(name="smalls", bufs=8))
    psA = ctx.enter_context(tc.tile_pool(name="psA", bufs=2, space="PSUM"))
    psO = ctx.enter_context(tc.tile_pool(name="psO", bufs=2, space="PSUM"))
    psS = ctx.enter_context(tc.tile_pool(name="psS", bufs=2, space="PSUM"))

    # ---- constants ----
    wqk_sb = consts.tile([128, 8, 128], FP16, tag="wqk")
    nc.sync.dma_start(out=wqk_sb, in_=wqkT.rearrange("(t p) m -> p t m", p=128))
    wv_sb = consts.tile([128, 8, 64], FP16, tag="wv")
    nc.sync.dma_start(out=wv_sb, in_=wvT.rearrange("(t p) m -> p t m", p=128))
    w1_sb = consts.tile([128, 1024], FP16, tag="w1")
    nc.sync.dma_start(out=w1_sb, in_=w1[:, :])
    w2_sb = consts.tile([128, 4, 1024], FP16, tag="w2")
    nc.sync.dma_start(out=w2_sb, in_=w2.rearrange("(t p) j -> p t j", p=128))
    bv_sb = consts.tile([1, 1024], FP16, tag="bv")
    nc.sync.dma_start(out=bv_sb, in_=bvec[:, :])
    ident = consts.tile([128, 128], FP16, tag="ident")
    make_identity(nc, ident)
    ones_sb = consts.tile([1, 128], FP16, tag="ones")
    nc.vector.memset(ones_sb, 1.0)

    # ---- persistent activations (all fp16, PSUM accumulation is fp32) ----
    qTb = [persist.tile([128, 1024], FP16, tag=f"qTb{i}", name=f"qTb{i}")
           for i in range(NPAIR)]
    kTb = [persist.tile([128, 1024], FP16, tag=f"kTb{i}", name=f"kTb{i}")
           for i in range(NPAIR)]
    for t in qTb + kTb:
        nc.vector.memset(t[64:128, :], 0.0)
    vT = [persist.tile([64, SB], FP16, tag=f"vT{i}", name=f"vT{i}")
          for i in range(NSB)]
    vaug = [persist.tile([128, 65], FP16, tag=f"va{i}", name=f"va{i}")
            for i in range(MT)]
    oT = [persist.tile([128, SB], FP16, tag=f"oT{i}", name=f"oT{i}")
          for i in range(NSB)]
    for t in oT:
        nc.vector.memset(t[64:128, :], 0.0)
    vflatT = [
        persist.tile([128, 128], FP16, tag=f"vf{i}", name=f"vf{i}")
        for i in range(16)
    ]

    # ---- qkv projection for one sblk pair ----
    def emit_qkv(pr):
        xts = []
        for c in range(8):
            xt = xpool.tile([128, 1024], FP16, name=f"xt{pr}_{c}", tag="xt")
            nc.sync.dma_start(
                out=xt, in_=xT[c * 128:(c + 1) * 128, pr * 1024:(pr + 1) * 1024]
            )
            xts.append(xt)
        qk_ps = psA.tile([128, 1024], FP32, name=f"qkps{pr}", tag="A")
        for c in range(8):
            for half in range(2):
                nc.tensor.matmul(
                    qk_ps[:, half * SB:(half + 1) * SB],
                    lhsT=wqk_sb[:, c, :],
                    rhs=xts[c][:, half * SB:(half + 1) * SB],
                    start=(c == 0), stop=(c == 7),
                )
        for half in range(2):
            sl = slice(half * SB, (half + 1) * SB)
            nc.vector.tensor_copy(qTb[pr][0:64, sl], qk_ps[0:64, sl])
            nc.vector.tensor_copy(kTb[pr][0:64, sl], qk_ps[64:128, sl])
        v_ps = psA.tile([64, 1024], FP32, name=f"vps{pr}", tag="A")
        for c in range(8):
            for half in range(2):
                nc.tensor.matmul(
                    v_ps[:, half * SB:(half + 1) * SB],
                    lhsT=wv_sb[:, c, :],
                    rhs=xts[c][:, half * SB:(half + 1) * SB],
                    start=(c == 0), stop=(c == 7),
                )
        for half in range(2):
            sblk = 2 * pr + half
            nc.vector.tensor_copy(vT[sblk], v_ps[:, half * SB:(half + 1) * SB])
            # v natural layout [m, d] (+ ones col) for the ov matmul
            for j in range(4):
                mt = sblk * 4 + j
                tr_ps = psS.tile([128, 64], FP16, name=f"vtr{mt}", tag="sp")
                nc.tensor.transpose(
                    tr_ps, vT[sblk][:, j * 128:(j + 1) * 128], ident[0:64, 0:64]
                )
                nc.vector.tensor_copy(vaug[mt][:, 0:64], tr_ps)
                nc.vector.memset(vaug[mt][:, 64:65], 1.0)

    # vflatT[c2, r] = v[8r + c2//64, c2%64] (feeds the proj directly)
    def emit_vflat():
        for k2 in range(4):
            for rt in range(4):
                vf = vflatT[k2 * 4 + rt]
                for s01 in range(2):
                    for half in range(2):
                        t = 2 * rt + half
                        src = vT[t].rearrange("p (r s) -> p s r", s=8)
                        nc.vector.tensor_copy(
                            vf[64 * s01:64 * s01 + 64, 64 * half:64 * half + 64],
                            src[:, 2 * k2 + s01, :],
                        )

    # ---- pending PE work (yv, yo) drip-fed into the attention pipeline ----
    pending = deque()

    def queue_yv():
        yvstage = [None] * 4

        yps_live = {}

        def mk_a(rt, half):
            def thunk():
                yps = psS.tile([128, SB], FP32, name=f"yvps{rt}_{half}", tag="sp")
                yps_live[(rt, half)] = yps
                for k2 in range(2):
                    nc.tensor.matmul(
                        yps, lhsT=vflatT[k2 * 4 + rt],
                        rhs=w2_sb[:, k2, half * 512:(half + 1) * 512],
                        start=(k2 == 0), stop=False,
                    )
            return thunk

        def mk_b(rt, half):
            def thunk():
                yps = yps_live.pop((rt, half))
                for k2 in range(2, 4):
                    nc.tensor.matmul(
                        yps, lhsT=vflatT[k2 * 4 + rt],
                        rhs=w2_sb[:, k2, half * 512:(half + 1) * 512],
                        start=False, stop=False,
                    )
                nc.tensor.matmul(
                    yps, lhsT=ones_sb,
                    rhs=bv_sb[:, half * 512:(half + 1) * 512],
                    start=False, stop=True,
                )
                if yvstage[rt] is None:
                    yvstage[rt] = ypool.tile(
                        [128, 1024], FP32, name=f"yvst{rt}", tag="yst"
                    )
                nc.vector.tensor_copy(
                    yvstage[rt][:, half * 512:(half + 1) * 512], yps
                )
                if half == 1:
                    nc.sync.dma_start(
                        out=yvout[rt * 128:(rt + 1) * 128, :], in_=yvstage[rt]
                    )
            return thunk

        for rt in range(4):
            for half in range(2):
                pending.append(mk_a(rt, half))
                pending.append(mk_b(rt, half))

    def queue_yo(pr):
        ystages = {}

        def mk(j, half):
            def thunk():
                if j not in ystages:
                    ystages[j] = ypool.tile(
                        [128, 1024], FP32, name=f"yst{pr}_{j}", tag="yst"
                    )
                ystage = ystages[j]
                nb = 2 * pr + half
                yps = psS.tile([128, SB], FP32, name=f"yops{pr}_{j}_{half}",
                               tag="sp")
                nc.tensor.matmul(
                    yps, lhsT=w1_sb[:, j * 128:(j + 1) * 128], rhs=oT[nb],
                    start=True, stop=True,
                )
                nc.vector.tensor_copy(
                    ystage[:, half * 512:(half + 1) * 512], yps
                )
                if half == 1:
                    nc.sync.dma_start(
                        out=ypartT[j * 128:(j + 1) * 128,
                                   pr * 1024:(pr + 1) * 1024],
                        in_=ystages.pop(j),
                    )
            return thunk

        for j in range(8):
            pending.append(mk(j, 0))
            pending.append(mk(j, 1))

    # ---- attention: dual-nblock sweeps, software-pipelined ----
    groups = [(pr, mt) for pr in range(NPAIR) for mt in range(MT)]
    s_ps = [None] * len(groups)
    p_sb = [None] * len(groups)
    oaug = [None] * NSB

    def emit_s(i):
        pr, mt = groups[i]
        ps = psA.tile([128, 1024], FP32, name=f"sps{pr}_{mt}", tag="A")
        lhsT = kTb[mt // 8][:, (mt % 8) * 128:(mt % 8) * 128 + 128]
        for half in range(2):
            sl = slice(half * SB, (half + 1) * SB)
            nc.tensor.matmul(ps[:, sl], lhsT=lhsT, rhs=qTb[pr][:, sl],
                             start=True, stop=True)
        s_ps[i] = ps

    def emit_exp(i):
        pr, mt = groups[i]
        pt = ppool.tile([128, 1024], FP16, name=f"pt{pr}_{mt}", tag="pt")
        nc.scalar.activation(pt, s_ps[i], Exp, scale=SCALE)
        p_sb[i] = pt

    def emit_ov(i):
        pr, mt = groups[i]
        for half in range(2):
            nb = 2 * pr + half
            if oaug[nb] is None:
                oaug[nb] = psO.tile([128, SB], FP32, name=f"oaug{nb}", tag="oaug")
            for nt in range(4):
                # one accumulation group per PSUM bank: single start/stop,
                # first touch of each byte overwrites via pending-zero bits
                nc.tensor.matmul(
                    oaug[nb][:, nt * 128:nt * 128 + 65],
                    lhsT=p_sb[i][:, half * SB + nt * 128:half * SB + nt * 128 + 128],
                    rhs=vaug[mt],
                    start=(mt == 0 and nt == 0), stop=(mt == MT - 1 and nt == 3),
                )
        p_sb[i] = None
        s_ps[i] = None

    def emit_tail(pr):
        for half in range(2):
            nb = 2 * pr + half
            for nt in range(4):
                sl = oaug[nb][:, nt * 128:nt * 128 + 65]
                rec = smalls.tile([128, 1], FP32, name=f"rec{nb}_{nt}", tag="rec")
                nc.vector.reciprocal(rec, sl[:, 64:65])
                onrm = smalls.tile([128, 64], FP16, name=f"on{nb}_{nt}", tag="onrm")
                nc.vector.tensor_scalar_mul(onrm, sl[:, 0:64], rec)
                tr_ps = psS.tile([64, 128], FP16, name=f"otr{nb}_{nt}", tag="sp")
                nc.tensor.transpose(tr_ps, onrm, ident)
                nc.vector.tensor_copy(oT[nb][0:64, nt * 128:nt * 128 + 128], tr_ps)
            oaug[nb] = None

    # emission schedule: qkv pairs 0-1 up front, pairs 2-3 and the
    # vflat/yv prep interleaved into pair-0's attention sweep
    emit_qkv(0)
    emit_s(0)
    emit_s(1)
    for i in range(len(groups)):
        if i + 2 < len(groups):
            emit_s(i + 2)
        emit_exp(i)
        emit_ov(i)
        if pending:
            pending.popleft()()
        pr, mt = groups[i]
        if pr == 0 and mt == 3:
            emit_qkv(1)
        elif pr == 0 and mt == 7:
            emit_qkv(2)
        elif pr == 0 and mt == 15:
            emit_qkv(3)
        elif pr == 0 and mt == 23:
            emit_vflat()
            queue_yv()
        if mt == MT - 1:
            emit_tail(pr)
            queue_yo(pr)
    while pending:
        pending.popleft()()


def build_program():
    nc = bacc.Bacc()
    xT = nc.declare_dram_parameter("xT", [1024, N], FP16, isOutput=False)
    wqkT = nc.declare_dram_parameter("wqkT", [1024, 128], FP16, isOutput=False)
    wvT = nc.declare_dram_parameter("wvT", [1024, 64], FP16, isOutput=False)
    w1 = nc.declare_dram_parameter("w1", [128, 1024], FP16, isOutput=False)
    w2 = nc.declare_dram_parameter("w2", [512, 1024], FP16, isOutput=False)
    bvec = nc.declare_dram_parameter("bvec", [1, 1024], FP16, isOutput=False)
    ypartT = nc.declare_dram_parameter("ypartT", [1024, N], FP32, isOutput=True)
    yvout = nc.declare_dram_parameter("yvout", [512, 1024], FP32, isOutput=True)
    io = (xT[:], wqkT[:], wvT[:], w1[:], w2[:], bvec[:], ypartT[:], yvout[:])
    with tile.TileContext(nc) as tc:
        with ExitStack() as ctx:
            _emit(ctx, tc, nc, io)
    nc.compile()
    return nc


def make_in_maps(query, value, w_qkv, w_proj, b_proj):
    x = np.concatenate([query[0], value[0]], axis=1).astype(np.float32)
    xT = np.ascontiguousarray(x.T).astype(np.float16)
    w2 = np.ascontiguousarray(w_proj[:, 512:1024].T).astype(np.float16)
    bv = np.ascontiguousarray(b_proj[None, :]).astype(np.float16)
    in_maps = []
    for h in range(H):
        wqk = np.ascontiguousarray(
            np.concatenate(
                [w_qkv[h * 64:(h + 1) * 64], w_qkv[512 + h * 64:512 + (h + 1) * 64]],
                axis=0,
            ).T
        ).astype(np.float16)
        wv = np.ascontiguousarray(
            w_qkv[1024 + h * 64:1024 + (h + 1) * 64].T
        ).astype(np.float16)
        w1 = np.zeros((128, 1024), np.float16)
        w1[:64] = w_proj[:, h * 64:(h + 1) * 64].T.astype(np.float16)
        in_maps.append(
            {"xT": xT, "wqkT": wqk, "wvT": wv, "w1": w1, "w2": w2, "bvec": bv}
        )
    return in_maps


def combine_results(results):
    yT = np.zeros((1024, N), np.float32)
    for h in range(H):
        yT += results[h]["ypartT"]
    y = np.ascontiguousarray(yT.T)
    for h in range(H):
        y[h * 512:(h + 1) * 512] += results[h]["yvout"]
    y0 = np.ascontiguousarray(y[:, :512].reshape(1, N, 512))
    y1 = np.ascontiguousarray(y[:, 512:].reshape(1, N, 512))
    return y0, y1


_PROGRAM = None


def kernel(query, value, w_qkv, w_proj, b_proj, **_):
    global _PROGRAM
    if _PROGRAM is None:
        _PROGRAM = build_program()
    in_maps = make_in_maps(query, value, w_qkv, w_proj, b_proj)
    res = run_bass_kernel_spmd(_PROGRAM, in_maps, list(range(H)))
    return combine_results(res.results)
